# revision 1
# baseline (speedup 1.0000x reference)
"""Baichuan paged-attention layer on 8 trn2 cores, tensor-parallel over heads.

Per core c: heads 4c..4c+3. Device computes QKV proj (f32r matmuls), RoPE,
attention vs [gathered history KV + new KV], and a partial o_proj
[T, HID] against w_o[:, 512c:512c+512]. Host gathers history KV pages,
builds RoPE/mask tables, and sums the 8 partial outputs. Weight/hidden
DMAs are chunked (multiple 128-row slabs per transfer) to keep the sync
sequencer's per-DMA trigger cost off the critical path.
"""
import sys

sys.path.insert(0, "/opt/trn_rl_repo")
import numpy as np

H = 32; D = 128; HID = 4096; BS = 64; NBLOCKS = 128
B = 4; QLEN = 512; MAXBLK = 24; ROPE_BASE = 10000.0
T = B * QLEN; NCORES = 8; HC = H // NCORES; W = HC * D  # 4 heads, 512 wide
NEG = -1.0e30
SCALE = 1.0 / float(np.sqrt(D))

_cache = {}
last_results = None  # BassKernelResults of the most recent run (for test.py)

# pool sizing knobs (tuned against the instruction cost model)
BUFS = dict(cs=1, hid=4, wq=4, wv=3, qkr=8, qs=2, ropet=2, vsb=4,
            kh=2, vh=1, exp=3, smol=1, stg=4, wo=3, attn=16)


def _round128(x):
    return (x + 127) // 128 * 128


def _build(hist):
    import concourse.bass as bass
    import concourse.tile as tile
    from concourse import bacc, mybir

    F32 = mybir.dt.float32
    F32R = mybir.dt.float32r

    hv = [_round128(h) for h in hist]
    SH = [x // 128 for x in hv]

    nc = bacc.Bacc("TRN2", target_bir_lowering=False, debug=False,
                   num_devices=NCORES)
    hiddenT_d = nc.dram_tensor("hiddenT", [HID, T], F32R, kind="ExternalInput")
    # wqr: [rt, kslab, p, col] flattened -> [8*HID, 128]
    wqr_d = nc.dram_tensor("wqr", [8 * HID, 128], F32R, kind="ExternalInput")
    wvT_d = nc.dram_tensor("wvT", [HID, W], F32R, kind="ExternalInput")
    woT_d = nc.dram_tensor("woT", [W, HID], F32R, kind="ExternalInput")
    kh_d = [nc.dram_tensor(f"khT{b}", [W, hv[b]], F32R, kind="ExternalInput")
            if hv[b] else None for b in range(B)]
    vh_d = [nc.dram_tensor(f"vh{b}", [hv[b], W], F32R, kind="ExternalInput")
            if hv[b] else None for b in range(B)]
    out_d = nc.dram_tensor("out", [T, HID], F32, kind="ExternalOutput")

    # host-built tables baked into the NEFF
    inv = 1.0 / (ROPE_BASE ** (np.arange(0, D, 2) / D))
    pos = np.concatenate([h + np.arange(QLEN) for h in hist]).astype(np.float64)
    ang = np.concatenate([inv, inv])[:, None] * pos[None, :]
    cos_d = nc.inline_tensor(np.cos(ang).astype(np.float32), name="cosT")
    sin_d = nc.inline_tensor(np.sin(ang).astype(np.float32), name="sinT")

    mask_np = np.where(
        np.arange(128)[:, None] <= np.arange(896)[None, :] - 384,
        0.0, NEG).astype(np.float32)
    mask_d = nc.inline_tensor(mask_np, name="maskS")

    pad_np = np.zeros((128, B), np.float32)
    for b in range(B):
        if hv[b]:
            pad_np[:, b] = np.where(hv[b] - 128 + np.arange(128) >= hist[b],
                                    NEG, 0.0)
    pad_d = nc.inline_tensor(pad_np, name="padc")

    Pm = np.zeros((128, 128), np.float32)
    for d in range(64):
        Pm[d, d + 64] = -1.0
        Pm[d + 64, d] = 1.0
    pt_d = nc.inline_tensor(np.ascontiguousarray(Pm.T), name="permT")
    ones_d = nc.inline_tensor(np.ones((128, 1), np.float32), name="ones")

    with tile.TileContext(nc) as tc:
        with tc.tile_pool(name="const", bufs=1) as cpool, \
             tc.tile_pool(name="attn", bufs=BUFS["attn"]) as apool, \
             tc.tile_pool(name="psum", bufs=8, space="PSUM") as pspool:
            mask_t = cpool.tile([128, 896], F32, tag="mask")
            nc.sync.dma_start(mask_t[:], mask_d[:])
            pad_t = cpool.tile([128, B], F32, tag="pad")
            nc.sync.dma_start(pad_t[:], pad_d[:])
            pt_t = cpool.tile([128, 128], F32R, tag="pt")
            nc.sync.dma_start(pt_t[:], pt_d[:].bitcast(F32R))
            ones_t = cpool.tile([128, 1], F32R, tag="ones")
            nc.sync.dma_start(ones_t[:], ones_d[:].bitcast(F32R))

            attn_sb = [[None] * HC for _ in range(B)]

            with tc.tile_pool(name="cs", bufs=BUFS["cs"]) as cspool, \
                 tc.tile_pool(name="hid", bufs=BUFS["hid"]) as hidpool, \
                 tc.tile_pool(name="wst", bufs=BUFS["wq"]) as wqpool, \
                 tc.tile_pool(name="wvst", bufs=BUFS["wv"]) as wvpool, \
                 tc.tile_pool(name="qkr", bufs=BUFS["qkr"]) as qkrpool, \
                 tc.tile_pool(name="rope", bufs=BUFS["qs"]) as rppool, \
                 tc.tile_pool(name="vsb", bufs=BUFS["vsb"]) as vpool, \
                 tc.tile_pool(name="khp", bufs=BUFS["kh"]) as khpool, \
                 tc.tile_pool(name="vhp", bufs=BUFS["vh"]) as vhpool, \
                 tc.tile_pool(name="expp", bufs=BUFS["exp"]) as epool, \
                 tc.tile_pool(name="smol", bufs=BUFS["smol"]) as smpool:
                for b in range(B):
                    tsl = slice(b * QLEN, (b + 1) * QLEN)
                    cos_t = cspool.tile([128, QLEN], F32, tag="cos")
                    nc.sync.dma_start(cos_t[:], cos_d[:, tsl])
                    sin_t = cspool.tile([128, QLEN], F32, tag="sin")
                    nc.sync.dma_start(sin_t[:], sin_d[:, tsl])

                    # hidden for this seq: 4 chunks of 8 k-slabs
                    hid_c = []
                    for kc in range(4):
                        ht = hidpool.tile([128, 8, QLEN], F32R, tag="hid",
                                          name=f"hid{b}_{kc}")
                        nc.sync.dma_start(
                            ht[:],
                            hiddenT_d[kc * 1024:(kc + 1) * 1024, tsl]
                            .rearrange("(s p) t -> p s t", p=128))
                        hid_c.append(ht)

                    # ---- QK projection + RoPE: qk_rot[rt] = [128 d, 512 t]
                    # V-projection chunks are interleaved between rt sweeps to
                    # spread the wv DMA stream over the whole seq.
                    v_sb = [vpool.tile([128, W], F32R, tag="vsb",
                                       name=f"vsb{b}_{i}") for i in range(4)]
                    v_ps = [pspool.tile([128, W], F32, tag="ps",
                                        name=f"vps{b}_{i}") for i in range(4)]
                    qk_rot = []
                    for rt in range(8):
                        pq = pspool.tile([128, QLEN], F32, tag="ps")
                        for kc in range(4):
                            wqt = wqpool.tile([128, 8, 128], F32R, tag="wq")
                            nc.sync.dma_start(
                                wqt[:],
                                wqr_d[rt * HID + kc * 1024:
                                      rt * HID + (kc + 1) * 1024, :]
                                .rearrange("(s p) c -> p s c", p=128))
                            for s in range(8):
                                nc.tensor.matmul(
                                    pq[:], wqt[:, s, :], hid_c[kc][:, s, :],
                                    start=(kc == 0 and s == 0),
                                    stop=(kc == 3 and s == 7))
                        qs = rppool.tile([128, QLEN], F32R, tag="qs")
                        nc.scalar.copy(qs[:], pq[:])
                        rot = pspool.tile([128, QLEN], F32, tag="ps")
                        nc.tensor.matmul(rot[:], pt_t[:], qs[:],
                                         start=True, stop=True)
                        t1 = rppool.tile([128, QLEN], F32, tag="t1",
                                         bufs=BUFS["ropet"])
                        nc.vector.tensor_mul(t1[:], rot[:], sin_t[:])
                        t2 = rppool.tile([128, QLEN], F32, tag="t2",
                                         bufs=BUFS["ropet"])
                        nc.vector.tensor_mul(t2[:], qs[:], cos_t[:])
                        qr = qkrpool.tile([128, QLEN], F32R, tag="qkr")
                        nc.vector.tensor_add(qr[:], t1[:], t2[:])
                        qk_rot.append(qr)
                        for kc2 in (2 * rt, 2 * rt + 1):
                            wvt = wvpool.tile([128, 2, W], F32R, tag="wv")
                            nc.sync.dma_start(
                                wvt[:],
                                wvT_d[kc2 * 256:(kc2 + 1) * 256, :]
                                .rearrange("(s p) c -> p s c", p=128))
                            for s2 in range(2):
                                k = kc2 * 2 + s2
                                for tt in range(4):
                                    nc.tensor.matmul(
                                        v_ps[tt][:],
                                        hid_c[k // 8][:, k % 8,
                                                      tt * 128:(tt + 1) * 128],
                                        wvt[:, s2, :],
                                        start=(k == 0), stop=(k == 31))
                    for tt in range(4):
                        nc.vector.tensor_copy(v_sb[tt][:], v_ps[tt][:])

                    # ---- history V: one chunked DMA per seq
                    vht = None
                    if SH[b]:
                        vht = vhpool.tile([128, SH[b], W], F32R, tag="vh",
                                          name=f"vh_t{b}")
                        nc.sync.dma_start(
                            vht[:],
                            vh_d[b][:].rearrange("(s p) c -> p s c", p=128))

                    # ---- attention per head
                    S = SH[b] + 4
                    for h in range(HC):
                        kh_t = None
                        if SH[b]:
                            kh_t = khpool.tile([128, hv[b]], F32R, tag="kh")
                            nc.sync.dma_start(
                                kh_t[:], kh_d[b][h * 128:(h + 1) * 128, :])
                        dn = pspool.tile([1, QLEN], F32, tag="ps")
                        pv = pspool.tile([128, QLEN], F32, tag="ps")
                        for st in range(S):
                            sc = pspool.tile([128, QLEN], F32, tag="ps")
                            if st < SH[b]:
                                lhsT = kh_t[:, st * 128:(st + 1) * 128]
                            else:
                                j = st - SH[b]
                                lhsT = qk_rot[4 + h][:, j * 128:(j + 1) * 128]
                            nc.tensor.matmul(sc[:], lhsT, qk_rot[h][:],
                                             start=True, stop=True)
                            if st == SH[b] - 1 and hist[b] != hv[b]:
                                nc.vector.tensor_scalar_add(
                                    sc[:], sc[:], pad_t[:, b:b + 1])
                            if st >= SH[b]:
                                j = st - SH[b]
                                nc.vector.tensor_add(
                                    sc[:], sc[:],
                                    mask_t[:, 384 - 128 * j:896 - 128 * j])
                            ex = epool.tile([128, QLEN], F32R, tag="exp")
                            nc.scalar.activation(
                                ex[:], sc[:], mybir.ActivationFunctionType.Exp,
                                scale=SCALE)
                            nc.tensor.matmul(dn[:], ones_t[:], ex[:],
                                             start=(st == 0), stop=(st == S - 1))
                            if st < SH[b]:
                                vt = vht[:, st, h * 128:(h + 1) * 128]
                            else:
                                vt = v_sb[st - SH[b]][:, h * 128:(h + 1) * 128]
                            nc.tensor.matmul(pv[:], vt, ex[:],
                                             start=(st == 0), stop=(st == S - 1))
                        rc = smpool.tile([1, QLEN], F32, tag="rc")
                        nc.vector.reciprocal(rc[:], dn[:])
                        bcs = smpool.tile([128, QLEN], F32, tag="bcs")
                        nc.gpsimd.partition_broadcast(bcs[:], rc[:])
                        at = apool.tile([128, QLEN], F32R, tag="attn")
                        nc.vector.tensor_mul(at[:], pv[:], bcs[:])
                        attn_sb[b][h] = at

            # ---- o_proj partial: ic-outer, wo streamed per 512-col chunk
            with tc.tile_pool(name="wop", bufs=8) as wopool, \
                 tc.tile_pool(name="stg", bufs=BUFS["stg"]) as stpool:
                wots = []
                for ic in range(8):
                    isl = slice(ic * 512, (ic + 1) * 512)
                    wot = wopool.tile([128, 4, 512], F32R, tag="wo",
                                      name=f"wot{ic}")
                    nc.sync.dma_start(
                        wot[:],
                        woT_d[:, isl].rearrange("(s p) c -> p s c", p=128))
                    wots.append(wot)
                for ic in range(8):
                    isl = slice(ic * 512, (ic + 1) * 512)
                    wot = wots[ic]
                    for tt in range(16):
                        b, q = tt // 4, tt % 4
                        po = pspool.tile([128, 512], F32, tag="ps")
                        for jt in range(4):
                            nc.tensor.matmul(
                                po[:],
                                attn_sb[b][jt][:, q * 128:(q + 1) * 128],
                                wot[:, jt, :], start=(jt == 0), stop=(jt == 3))
                        st_ = stpool.tile([128, 512], F32, tag="stg")
                        if tt % 2 == 0:
                            nc.vector.tensor_copy(st_[:], po[:])
                        else:
                            nc.scalar.copy(st_[:], po[:])
                        nc.sync.dma_start(
                            out_d[tt * 128:(tt + 1) * 128, isl], st_[:])
    nc.compile()
    return {"nc": nc}


def _get(hist):
    if hist not in _cache:
        _cache[hist] = _build(hist)
    return _cache[hist]


def prepare_in_maps(inputs):
    hidden = np.asarray(inputs["hidden_states"], np.float32)
    w_pack = np.asarray(inputs["w_pack"], np.float32)
    w_o = np.asarray(inputs["w_o"], np.float32)
    kc = np.asarray(inputs["key_cache"], np.float32).reshape(NBLOCKS * BS, H, D)
    vc = np.asarray(inputs["value_cache"], np.float32).reshape(NBLOCKS * BS, H, D)
    bo = np.asarray(inputs["block_offsets"], np.int32)
    hist = tuple(int(x) for x in np.asarray(inputs["history_lengths"]))
    assert all(0 <= h and h + QLEN <= MAXBLK * BS for h in hist)
    hv = [_round128(h) for h in hist]

    built = _get(hist)
    hiddenT = np.ascontiguousarray(hidden.T)

    in_maps = []
    for c in range(NCORES):
        rs = slice(c * W, (c + 1) * W)
        wqk = np.concatenate(
            [w_pack[rs], w_pack[HID + c * W:HID + (c + 1) * W]], axis=0)
        # wqr[rt, s, p, col] = wqk[rt*128+col, s*128+p]
        wqr = np.ascontiguousarray(
            wqk.reshape(8, 128, 32, 128).transpose(0, 2, 3, 1)
            .reshape(8 * HID, 128))
        wv = w_pack[2 * HID + c * W:2 * HID + (c + 1) * W]
        im = {
            "hiddenT": hiddenT,
            "wqr": wqr,
            "wvT": np.ascontiguousarray(wv.T),
            "woT": np.ascontiguousarray(w_o[:, rs].T),
        }
        for b in range(B):
            if not hv[b]:
                continue
            nblk = (hist[b] + BS - 1) // BS
            rows = (bo[b, :nblk, None] * BS +
                    np.arange(BS)[None, :]).reshape(-1)[:hist[b]]
            khp = np.zeros((hv[b], HC, D), np.float32)
            khp[:hist[b]] = kc[rows][:, c * HC:(c + 1) * HC, :]
            vhp = np.zeros((hv[b], HC, D), np.float32)
            vhp[:hist[b]] = vc[rows][:, c * HC:(c + 1) * HC, :]
            im[f"khT{b}"] = np.ascontiguousarray(
                khp.transpose(1, 2, 0).reshape(W, hv[b]))
            im[f"vh{b}"] = np.ascontiguousarray(vhp.reshape(hv[b], W))
        in_maps.append(im)
    return built["nc"], in_maps


def kernel(**inputs):
    global last_results
    from concourse.bass_utils import run_bass_kernel_spmd

    nc, in_maps = prepare_in_maps(inputs)
    last_results = run_bass_kernel_spmd(nc, in_maps,
                                        core_ids=list(range(NCORES)))
    acc = np.zeros((T, HID), np.float64)
    for c in range(NCORES):
        acc += last_results.results[c]["out"]
    return acc.astype(np.float32)



# revision 2
# speedup vs baseline: 2.6643x; 2.6643x over previous
"""Baichuan paged-attention layer on 8 trn2 cores, tensor-parallel over heads.

Per core c: heads 4c..4c+3. Device computes QKV proj, RoPE, attention vs
[gathered history KV + new KV], and a partial o_proj [T, HID] against
w_o[:, 512c:512c+512]. Host gathers history KV pages, builds RoPE/mask
tables, and sums the 8 partial outputs (bf16 partials, f64 accumulate).
All matmul operands are bf16 (fp32 PSUM accumulation); softmax/RoPE
arithmetic stays fp32 on the vector/scalar engines.
"""
import sys

sys.path.insert(0, "/opt/trn_rl_repo")
import numpy as np

H = 32; D = 128; HID = 4096; BS = 64; NBLOCKS = 128
B = 4; QLEN = 512; MAXBLK = 24; ROPE_BASE = 10000.0
T = B * QLEN; NCORES = 8; HC = H // NCORES; W = HC * D  # 4 heads, 512 wide
NEG = -1.0e30
SCALE = 1.0 / float(np.sqrt(D))

_cache = {}
last_results = None  # BassKernelResults of the most recent run (for test.py)

# pool sizing knobs
BUFS = dict(cs=1, hid=4, wq=4, wv=3, qkr=8, qs=2, ropet=2, vsb=4,
            kh=2, vh=1, exp=3, smol=1, stg=4, wo=3, attn=16)


def _round128(x):
    return (x + 127) // 128 * 128


def _np_bf16():
    from concourse import mybir
    return mybir.dt.np(mybir.dt.bfloat16)


def _build(hist):
    import concourse.bass as bass
    import concourse.tile as tile
    from concourse import bacc, mybir

    F32 = mybir.dt.float32
    BF16 = mybir.dt.bfloat16
    np_bf16 = _np_bf16()

    hv = [_round128(h) for h in hist]
    SH = [x // 128 for x in hv]

    nc = bacc.Bacc("TRN2", target_bir_lowering=False, debug=False,
                   num_devices=NCORES)
    hiddenT_d = nc.dram_tensor("hiddenT", [HID, T], BF16, kind="ExternalInput")
    # wql: [p, (rt*4+kc)*1024 + s*128 + c] = wqk[rt*128+c, kc*1024+s*128+p]
    wql_d = nc.dram_tensor("wql", [128, 8 * 4 * 1024], BF16,
                           kind="ExternalInput")
    wvT_d = nc.dram_tensor("wvT", [HID, W], BF16, kind="ExternalInput")
    woT_d = nc.dram_tensor("woT", [W, HID], BF16, kind="ExternalInput")
    kh_d = [nc.dram_tensor(f"khT{b}", [W, hv[b]], BF16, kind="ExternalInput")
            if hv[b] else None for b in range(B)]
    vh_d = [nc.dram_tensor(f"vh{b}", [hv[b], W], BF16, kind="ExternalInput")
            if hv[b] else None for b in range(B)]
    out_d = nc.dram_tensor("out", [T, HID], BF16, kind="ExternalOutput")

    # host-built tables baked into the NEFF
    inv = 1.0 / (ROPE_BASE ** (np.arange(0, D, 2) / D))
    pos = np.concatenate([h + np.arange(QLEN) for h in hist]).astype(np.float64)
    ang = np.concatenate([inv, inv])[:, None] * pos[None, :]
    cos_d = nc.inline_tensor(np.cos(ang).astype(np.float32), name="cosT")
    sin_d = nc.inline_tensor(np.sin(ang).astype(np.float32), name="sinT")

    mask_np = np.where(
        np.arange(128)[:, None] <= np.arange(896)[None, :] - 384,
        0.0, NEG).astype(np.float32)
    mask_d = nc.inline_tensor(mask_np, name="maskS")

    pad_np = np.zeros((128, B), np.float32)
    for b in range(B):
        if hv[b]:
            pad_np[:, b] = np.where(hv[b] - 128 + np.arange(128) >= hist[b],
                                    NEG, 0.0)
    pad_d = nc.inline_tensor(pad_np, name="padc")

    Pm = np.zeros((128, 128), np.float32)
    for d in range(64):
        Pm[d, d + 64] = -1.0
        Pm[d + 64, d] = 1.0
    pt_d = nc.inline_tensor(np.ascontiguousarray(Pm.T).astype(np_bf16),
                            name="permT")
    ones_d = nc.inline_tensor(np.ones((128, 1), np_bf16), name="ones")

    with tile.TileContext(nc) as tc:
        with tc.tile_pool(name="const", bufs=1) as cpool, \
             tc.tile_pool(name="attn", bufs=BUFS["attn"]) as apool, \
             tc.tile_pool(name="psum", bufs=8, space="PSUM") as pspool:
            mask_t = cpool.tile([128, 896], F32, tag="mask")
            nc.sync.dma_start(mask_t[:], mask_d[:])
            pad_t = cpool.tile([128, B], F32, tag="pad")
            nc.sync.dma_start(pad_t[:], pad_d[:])
            pt_t = cpool.tile([128, 128], BF16, tag="pt")
            nc.sync.dma_start(pt_t[:], pt_d[:])
            ones_t = cpool.tile([128, 1], BF16, tag="ones")
            nc.sync.dma_start(ones_t[:], ones_d[:])

            attn_sb = [[None] * HC for _ in range(B)]

            with tc.tile_pool(name="cs", bufs=BUFS["cs"]) as cspool, \
                 tc.tile_pool(name="hid", bufs=BUFS["hid"]) as hidpool, \
                 tc.tile_pool(name="wst", bufs=BUFS["wq"]) as wqpool, \
                 tc.tile_pool(name="wvst", bufs=BUFS["wv"]) as wvpool, \
                 tc.tile_pool(name="qkr", bufs=BUFS["qkr"]) as qkrpool, \
                 tc.tile_pool(name="rope", bufs=BUFS["qs"]) as rppool, \
                 tc.tile_pool(name="vsb", bufs=BUFS["vsb"]) as vpool, \
                 tc.tile_pool(name="khp", bufs=BUFS["kh"]) as khpool, \
                 tc.tile_pool(name="vhp", bufs=BUFS["vh"]) as vhpool, \
                 tc.tile_pool(name="expp", bufs=BUFS["exp"]) as epool, \
                 tc.tile_pool(name="smol", bufs=BUFS["smol"]) as smpool:
                for b in range(B):
                    tsl = slice(b * QLEN, (b + 1) * QLEN)
                    cos_t = cspool.tile([128, QLEN], F32, tag="cos")
                    nc.sync.dma_start(cos_t[:], cos_d[:, tsl])
                    sin_t = cspool.tile([128, QLEN], F32, tag="sin")
                    nc.sync.dma_start(sin_t[:], sin_d[:, tsl])

                    # hidden for this seq: 4 chunks of 8 k-slabs
                    hid_c = []
                    for kc in range(4):
                        ht = hidpool.tile([128, 8, QLEN], BF16, tag="hid",
                                          name=f"hid{b}_{kc}")
                        nc.sync.dma_start(
                            ht[:],
                            hiddenT_d[kc * 1024:(kc + 1) * 1024, tsl]
                            .rearrange("(s p) t -> p s t", p=128))
                        hid_c.append(ht)

                    # ---- QK projection + RoPE: qk_rot[rt] = [128 d, 512 t]
                    # V-projection chunks are interleaved between rt sweeps to
                    # spread the wv DMA stream over the whole seq.
                    v_sb = [vpool.tile([128, W], BF16, tag="vsb",
                                       name=f"vsb{b}_{i}") for i in range(4)]
                    v_ps = [pspool.tile([128, W], F32, tag="ps",
                                        name=f"vps{b}_{i}") for i in range(4)]
                    qk_rot = []
                    for rt in range(8):
                        pq = pspool.tile([128, QLEN], F32, tag="ps")
                        for kc in range(4):
                            wqt = wqpool.tile([128, 1024], BF16, tag="wq")
                            nc.sync.dma_start(
                                wqt[:],
                                wql_d[:, (rt * 4 + kc) * 1024:
                                      (rt * 4 + kc + 1) * 1024])
                            for s in range(8):
                                nc.tensor.matmul(
                                    pq[:], wqt[:, s * 128:(s + 1) * 128],
                                    hid_c[kc][:, s, :],
                                    start=(kc == 0 and s == 0),
                                    stop=(kc == 3 and s == 7))
                        qs = rppool.tile([128, QLEN], BF16, tag="qs")
                        nc.scalar.copy(qs[:], pq[:])
                        rot = pspool.tile([128, QLEN], F32, tag="ps")
                        nc.tensor.matmul(rot[:], pt_t[:], qs[:],
                                         start=True, stop=True)
                        t1 = rppool.tile([128, QLEN], F32, tag="t1",
                                         bufs=BUFS["ropet"])
                        nc.vector.tensor_mul(t1[:], rot[:], sin_t[:])
                        t2 = rppool.tile([128, QLEN], F32, tag="t2",
                                         bufs=BUFS["ropet"])
                        nc.vector.tensor_mul(t2[:], pq[:], cos_t[:])
                        qr = qkrpool.tile([128, QLEN], BF16, tag="qkr")
                        nc.vector.tensor_add(qr[:], t1[:], t2[:])
                        qk_rot.append(qr)
                        for kc2 in (2 * rt, 2 * rt + 1):
                            wvt = wvpool.tile([128, 2, W], BF16, tag="wv")
                            nc.sync.dma_start(
                                wvt[:],
                                wvT_d[kc2 * 256:(kc2 + 1) * 256, :]
                                .rearrange("(s p) c -> p s c", p=128))
                            for s2 in range(2):
                                k = kc2 * 2 + s2
                                for tt in range(4):
                                    nc.tensor.matmul(
                                        v_ps[tt][:],
                                        hid_c[k // 8][:, k % 8,
                                                      tt * 128:(tt + 1) * 128],
                                        wvt[:, s2, :],
                                        start=(k == 0), stop=(k == 31))
                    for tt in range(4):
                        nc.vector.tensor_copy(v_sb[tt][:], v_ps[tt][:])

                    # ---- history V: one chunked DMA per seq
                    vht = None
                    if SH[b]:
                        vht = vhpool.tile([128, SH[b], W], BF16, tag="vh",
                                          name=f"vh_t{b}")
                        nc.sync.dma_start(
                            vht[:],
                            vh_d[b][:].rearrange("(s p) c -> p s c", p=128))

                    # ---- attention per head
                    S = SH[b] + 4
                    for h in range(HC):
                        kh_t = None
                        if SH[b]:
                            kh_t = khpool.tile([128, hv[b]], BF16, tag="kh")
                            nc.sync.dma_start(
                                kh_t[:], kh_d[b][h * 128:(h + 1) * 128, :])
                        dn = pspool.tile([1, QLEN], F32, tag="ps")
                        pv = pspool.tile([128, QLEN], F32, tag="ps")
                        for st in range(S):
                            sc = pspool.tile([128, QLEN], F32, tag="ps")
                            if st < SH[b]:
                                lhsT = kh_t[:, st * 128:(st + 1) * 128]
                            else:
                                j = st - SH[b]
                                lhsT = qk_rot[4 + h][:, j * 128:(j + 1) * 128]
                            nc.tensor.matmul(sc[:], lhsT, qk_rot[h][:],
                                             start=True, stop=True)
                            if st == SH[b] - 1 and hist[b] != hv[b]:
                                nc.vector.tensor_scalar_add(
                                    sc[:], sc[:], pad_t[:, b:b + 1])
                            if st >= SH[b]:
                                j = st - SH[b]
                                nc.vector.tensor_add(
                                    sc[:], sc[:],
                                    mask_t[:, 384 - 128 * j:896 - 128 * j])
                            ex = epool.tile([128, QLEN], BF16, tag="exp")
                            nc.scalar.activation(
                                ex[:], sc[:], mybir.ActivationFunctionType.Exp,
                                scale=SCALE)
                            nc.tensor.matmul(dn[:], ones_t[:], ex[:],
                                             start=(st == 0), stop=(st == S - 1))
                            if st < SH[b]:
                                vt = vht[:, st, h * 128:(h + 1) * 128]
                            else:
                                vt = v_sb[st - SH[b]][:, h * 128:(h + 1) * 128]
                            nc.tensor.matmul(pv[:], vt, ex[:],
                                             start=(st == 0), stop=(st == S - 1))
                        rc = smpool.tile([1, QLEN], F32, tag="rc")
                        nc.vector.reciprocal(rc[:], dn[:])
                        bcs = smpool.tile([128, QLEN], F32, tag="bcs")
                        nc.gpsimd.partition_broadcast(bcs[:], rc[:])
                        at = apool.tile([128, QLEN], BF16, tag="attn")
                        nc.vector.tensor_mul(at[:], pv[:], bcs[:])
                        attn_sb[b][h] = at

            # ---- o_proj partial: ic-outer, wo streamed per 512-col chunk
            with tc.tile_pool(name="wop", bufs=8) as wopool, \
                 tc.tile_pool(name="stg", bufs=BUFS["stg"]) as stpool:
                wots = []
                for ic in range(8):
                    isl = slice(ic * 512, (ic + 1) * 512)
                    wot = wopool.tile([128, 4, 512], BF16, tag="wo",
                                      name=f"wot{ic}")
                    nc.sync.dma_start(
                        wot[:],
                        woT_d[:, isl].rearrange("(s p) c -> p s c", p=128))
                    wots.append(wot)
                for ic in range(8):
                    isl = slice(ic * 512, (ic + 1) * 512)
                    wot = wots[ic]
                    for tt in range(16):
                        b, q = tt // 4, tt % 4
                        po = pspool.tile([128, 512], F32, tag="ps")
                        for jt in range(4):
                            nc.tensor.matmul(
                                po[:],
                                attn_sb[b][jt][:, q * 128:(q + 1) * 128],
                                wot[:, jt, :], start=(jt == 0), stop=(jt == 3))
                        st_ = stpool.tile([128, 512], BF16, tag="stg")
                        if tt % 2 == 0:
                            nc.vector.tensor_copy(st_[:], po[:])
                        else:
                            nc.scalar.copy(st_[:], po[:])
                        nc.sync.dma_start(
                            out_d[tt * 128:(tt + 1) * 128, isl], st_[:])
    nc.compile()
    return {"nc": nc}


def _get(hist):
    if hist not in _cache:
        _cache[hist] = _build(hist)
    return _cache[hist]


def prepare_in_maps(inputs):
    np_bf16 = _np_bf16()
    hidden = np.asarray(inputs["hidden_states"], np.float32)
    w_pack = np.asarray(inputs["w_pack"], np.float32)
    w_o = np.asarray(inputs["w_o"], np.float32)
    kc = np.asarray(inputs["key_cache"], np.float32).reshape(NBLOCKS * BS, H, D)
    vc = np.asarray(inputs["value_cache"], np.float32).reshape(NBLOCKS * BS, H, D)
    bo = np.asarray(inputs["block_offsets"], np.int32)
    hist = tuple(int(x) for x in np.asarray(inputs["history_lengths"]))
    assert all(0 <= h and h + QLEN <= MAXBLK * BS for h in hist)
    hv = [_round128(h) for h in hist]

    built = _get(hist)
    hiddenT = np.ascontiguousarray(hidden.T).astype(np_bf16)

    in_maps = []
    for c in range(NCORES):
        rs = slice(c * W, (c + 1) * W)
        wqk = np.concatenate(
            [w_pack[rs], w_pack[HID + c * W:HID + (c + 1) * W]], axis=0)
        # wql[p, rt, kc, s, c] = wqk[rt*128+c, kc*1024+s*128+p]
        wql = np.ascontiguousarray(
            wqk.reshape(8, 128, 4, 8, 128).transpose(4, 0, 2, 3, 1)
            .reshape(128, 8 * 4 * 1024)).astype(np_bf16)
        wv = w_pack[2 * HID + c * W:2 * HID + (c + 1) * W]
        im = {
            "hiddenT": hiddenT,
            "wql": wql,
            "wvT": np.ascontiguousarray(wv.T).astype(np_bf16),
            "woT": np.ascontiguousarray(w_o[:, rs].T).astype(np_bf16),
        }
        for b in range(B):
            if not hv[b]:
                continue
            nblk = (hist[b] + BS - 1) // BS
            rows = (bo[b, :nblk, None] * BS +
                    np.arange(BS)[None, :]).reshape(-1)[:hist[b]]
            khp = np.zeros((hv[b], HC, D), np.float32)
            khp[:hist[b]] = kc[rows][:, c * HC:(c + 1) * HC, :]
            vhp = np.zeros((hv[b], HC, D), np.float32)
            vhp[:hist[b]] = vc[rows][:, c * HC:(c + 1) * HC, :]
            im[f"khT{b}"] = np.ascontiguousarray(
                khp.transpose(1, 2, 0).reshape(W, hv[b])).astype(np_bf16)
            im[f"vh{b}"] = np.ascontiguousarray(
                vhp.reshape(hv[b], W)).astype(np_bf16)
        in_maps.append(im)
    return built["nc"], in_maps


def kernel(**inputs):
    global last_results
    from concourse.bass_utils import run_bass_kernel_spmd

    nc, in_maps = prepare_in_maps(inputs)
    last_results = run_bass_kernel_spmd(nc, in_maps,
                                        core_ids=list(range(NCORES)))
    acc = np.zeros((T, HID), np.float64)
    for c in range(NCORES):
        acc += last_results.results[c]["out"].astype(np.float64)
    return acc.astype(np.float32)
